# revision 2
# baseline (speedup 1.0000x reference)
"""CPC / NT-Xent loss kernel — symmetric (banded) Gram, 8 TRN2 cores.

Math (x, y: [8192, 256] f32):
    B = concat(x, y) row-L2-normalized           # [16384, 256]
    sim = B @ B.T ; E = exp(sim / tau), tau=0.5
    denom_r = sum_s E[r, s] - e^2
    pos_r   = dot(B_r, B_{r+8192 mod 16384})
    loss = mean(ln(denom) - 2 * pos)

E is symmetric: each unordered row pair is computed ONCE.  At 128-row
tile granularity (T=128 tiles), core c (input host-rotated by 2048c so
its rows are local tiles 0..15) computes, for each of its row tiles
i in 0..15:
  - band tiles (i, i+d), d = 0..63  -> global distance classes 0..63,
    each global ordered pair (A, A+d) covered exactly once across cores
  - the d=64 tile (i, i+64)         -> each unordered pair {A, A+64}
    covered exactly twice (once from each side), so it contributes
    ROW sums only - exact, no halving needed.
Row sums of exp come free via ACT accum_out.  Column sums (the
transposed pair's row-sum contribution) are accumulated on the DVE as
bf16 adds into `cs`; the 128-partition reduction and the cross-core
combine happen on the host (tiny numpy).

Schedule: column-chunk pass q outer, row tile i inner — pass q only
touches column tiles [i+16q, i+16q+16), so load slab s is first needed
at pass q = s-1.  Slab DVE work (sums of squares, Newton rsqrt,
normalize to bf16) is split into 4-tile units threaded between chunk
emissions of the previous pass, keeping the DVE from ever bursting
ahead of the ACT exp stream.  Row transposition is done by the DMA
XBAR (dma_start_transpose), not the PE: the PE runs matmuls only.

Host: combines rs/rs64/pos/cs from 8 cores, subtracts e^2, takes mean.
"""

import numpy as np
from contextlib import ExitStack

import concourse.bacc as bacc
import concourse.bass as bass
import concourse.tile as tile
import concourse.mybir as mybir
from concourse import bass_utils

F32 = mybir.dt.float32
BF16 = mybir.dt.bfloat16
FP8 = mybir.dt.float8e4
AF = mybir.ActivationFunctionType
ALU = mybir.AluOpType
DR = mybir.MatmulPerfMode.DoubleRow

P = 128
TAU = 0.5
N_CORES = 8

B_ROWS = 8192
H = 256                      # feature dim = 2 k-halves of 128
N_TOTAL = 2 * B_ROWS         # 16384
N_MINE = N_TOTAL // N_CORES  # 2048
NEWTON_ITERS = 5


class _Ctx:
    pass


def build_program(n_total=N_TOTAL, n_mine=N_MINE, chunk=2048,
                  enable_asserts=False, repeat=1,
                  ablate_colsum=False, ablate_exp_cols=None,
                  num_passes=None, fp8=True):
    T = n_total // P             # total row tiles (128)
    MT = n_mine // P             # my row tiles (16)
    D = T // 2                   # band width in tiles (64)
    LT = MT + D                  # tiles loaded locally (80)
    NQ = (D * P) // chunk        # column chunks per row tile (4)
    TPG = 16 if LT % 16 == 0 else LT   # tiles per load slab
    G = LT // TPG                # load slabs (5)
    assert LT % TPG == 0 and (D * P) % chunk == 0 and chunk % 512 == 0
    assert H == 2 * P

    nc = bacc.Bacc("TRN2", target_bir_lowering=False, debug=False,
                   enable_asserts=enable_asserts, num_devices=N_CORES)
    # bf16 input (host pre-casts): halves load DMA and puts the DVE's
    # square/normalize ops into the 2-byte 2x mode.
    b_dram = nc.dram_tensor("b", [n_total, H], BF16, kind="ExternalInput")
    rs_dram = nc.dram_tensor("rs", [P, MT * NQ], F32, kind="ExternalOutput")
    rs64_dram = nc.dram_tensor("rs64", [P, MT], F32, kind="ExternalOutput")
    pos_dram = nc.dram_tensor("pos", [P, MT], F32, kind="ExternalOutput")
    cs_dram = nc.dram_tensor("cs", [1, LT * P], F32, kind="ExternalOutput")

    with ExitStack() as ctx:
        tc = ctx.enter_context(tile.TileContext(nc))
        c = _Ctx()
        c.nc, c.b_ap = nc, b_dram.ap()
        c.T, c.MT, c.D, c.LT, c.NQ, c.TPG, c.G = T, MT, D, LT, NQ, TPG, G
        c.chunk, c.half = chunk, D
        c.rs_dram, c.rs64_dram = rs_dram, rs64_dram
        c.pos_dram, c.cs_dram = pos_dram, cs_dram
        c.ablate_colsum = ablate_colsum
        c.ablate_exp_cols = ablate_exp_cols
        c.num_passes = NQ if num_passes is None else num_passes
        c.fp8 = fp8

        bt_pool = ctx.enter_context(tc.tile_pool(name="bt", bufs=1))
        stat_pool = ctx.enter_context(tc.tile_pool(name="stat", bufs=1))
        c.load_pool = ctx.enter_context(tc.tile_pool(name="load", bufs=G))
        c.nrm_pool = ctx.enter_context(tc.tile_pool(name="nrm", bufs=2))
        c.keep_pool = ctx.enter_context(tc.tile_pool(name="keep", bufs=1))
        c.sq_pool = ctx.enter_context(tc.tile_pool(name="sq", bufs=4))
        c.nwt_pool = ctx.enter_context(tc.tile_pool(name="nwt", bufs=4))
        c.exp_pool = ctx.enter_context(tc.tile_pool(name="expo", bufs=8))
        c.cso_pool = ctx.enter_context(tc.tile_pool(name="cso", bufs=2))
        c.psum_pool = ctx.enter_context(tc.tile_pool(name="ps", bufs=2,
                                                     space="PSUM"))

        # B_T: normalized rows, transposed, bf16; one tile per k-half.
        c.BT0 = bt_pool.tile([P, LT * P], BF16, name="bt0")
        c.BT1 = bt_pool.tile([P, LT * P], BF16, name="bt1")
        # fp8 copy, k-halves as the middle dim - the exact operand layout
        # DoubleRow matmuls contract over ([K=128] x [2])
        c.BTf = bt_pool.tile([P, 2, LT * P], FP8, name="btf") if fp8 else None
        # colsum accumulator, bf16 (2x DVE mode); zeroed by gpsimd.
        c.acc = bt_pool.tile([P, LT * P], BF16, name="acc")

        c.ones = stat_pool.tile([P, 1], BF16)        # colsum-reduce lhsT
        c.ss_all = stat_pool.tile([P, LT], F32)      # row sums of squares
        c.inv_all = stat_pool.tile([P, LT], F32)     # 1/norm
        c.rs_all = stat_pool.tile([P, MT * NQ], F32)
        c.rs64 = stat_pool.tile([P, MT], F32)
        c.posA = stat_pool.tile([P, MT], F32)
        c.pos_all = stat_pool.tile([P, MT], F32)

        nc.vector.memset(c.ones[:], 1.0)
        for _rep in range(repeat):
            nc.gpsimd.memset(c.acc[:], 0.0)
            emit_all(c)

    nc.compile()
    return nc, "b", ("rs", "rs64", "pos", "cs")


def emit_load(c, g, nparts=1):
    """DMA slab g (dispatched on SP), optionally in `nparts` pieces so
    early processing can start before the whole slab lands."""
    nc = c.nc
    slab = c.load_pool.tile([P, c.TPG, H], BF16, tag="raw", name=f"slab{g}")
    tn = c.TPG // nparts
    for pt in range(nparts):
        t0 = pt * tn
        src = c.b_ap[(g * c.TPG + t0) * P:(g * c.TPG + t0 + tn) * P,
                     :].rearrange("(t p) m -> p t m", p=P)
        nc.sync.dma_start(out=slab[:, t0:t0 + tn, :], in_=src)
    c.slabs[g] = slab


def emit_sq(c, g, t0, tn):
    """Sums of squares for tiles [t0, t0+tn) of slab g.  On the ACT for
    bf16 builds (Square coexists with Exp in every act table set); on
    the DVE for fp8 builds, where the ACT also carries the fp8 casts."""
    nc = c.nc
    slab = c.slabs[g]
    for t in range(t0, t0 + tn):
        ti = g * c.TPG + t
        if c.fp8:
            sq = c.sq_pool.tile([P, H], BF16, tag="sqd", name="sqd")
            nc.vector.scalar_tensor_tensor(
                out=sq[:], in0=slab[:, t, :], scalar=1.0,
                in1=slab[:, t, :], op0=ALU.mult, op1=ALU.mult,
                accum_out=c.ss_all[:, ti:ti + 1])
        else:
            sq = c.sq_pool.tile([P, H], BF16, tag="sqa", name="sqa")
            nc.scalar.activation(
                out=sq[:], in_=slab[:, t, :], func=AF.Square,
                accum_out=c.ss_all[:, ti:ti + 1])


def emit_norm(c, g, t0, tn):
    """Newton rsqrt (one batched chain) + normalize for tiles
    [t0, t0+tn) of slab g (DVE)."""
    nc = c.nc
    slab = c.slabs[g]
    i0 = g * c.TPG + t0
    u_ss = c.ss_all[:, i0:i0 + tn]
    y0 = float(H) ** -0.5
    y = c.nwt_pool.tile([P, tn], F32, tag="nwty", name="nwty")
    nc.vector.tensor_scalar(
        out=y[:], in0=u_ss, scalar1=-0.5 * y0 ** 3, scalar2=1.5 * y0,
        op0=ALU.mult, op1=ALU.add)
    inv_slice = c.inv_all[:, i0:i0 + tn]
    for it in range(NEWTON_ITERS - 1):
        t1 = c.nwt_pool.tile([P, tn], F32, tag="nwtt", name="nwtt")
        nc.vector.scalar_tensor_tensor(
            out=t1[:], in0=y[:], scalar=1.0, in1=y[:],
            op0=ALU.mult, op1=ALU.mult)
        t2 = c.nwt_pool.tile([P, tn], F32, tag="nwtt2", name="nwtt2")
        nc.vector.scalar_tensor_tensor(
            out=t2[:], in0=u_ss, scalar=-0.5, in1=t1[:],
            op0=ALU.mult, op1=ALU.mult)
        last = it == NEWTON_ITERS - 2
        ynew = inv_slice if last else c.nwt_pool.tile(
            [P, tn], F32, tag="nwty", name="nwty")
        nc.vector.scalar_tensor_tensor(
            out=ynew if last else ynew[:], in0=t2[:], scalar=1.5, in1=y[:],
            op0=ALU.add, op1=ALU.mult)
        y = None if last else ynew

    nrm = c.nrms[g]
    for t in range(t0, t0 + tn):
        ti = g * c.TPG + t
        iv = c.inv_all[:, ti:ti + 1]
        # both k-halves in one op: out [P, 2, P] strided into the slab
        nc.vector.tensor_scalar_mul(
            nrm[:, :, t * P:(t + 1) * P],
            slab[:, t, :].rearrange("p (h q) -> p h q", h=2), iv)
        # pos: partner of my row-tile m is tile m + D
        if c.half <= ti < c.half + c.MT:
            m = ti - c.half
            for h, dst in ((0, c.posA), (1, c.pos_all)):
                sq2 = c.sq_pool.tile([P, P], BF16, tag="sq2", name="sq2")
                nc.vector.scalar_tensor_tensor(
                    out=sq2[:], in0=nrm[:, h, t * P:(t + 1) * P], scalar=1.0,
                    in1=c.kept[:, h, m * P:(m + 1) * P],
                    op0=ALU.mult, op1=ALU.mult,
                    accum_out=dst[:, m:m + 1])


def emit_xbar(c, g, part, nparts):
    """XBAR-transpose sub-range `part` of slab g's normalized halves
    into BT0/BT1 (dispatched on SP; waits on the DVE normalizes)."""
    nc = c.nc
    tn = c.TPG // nparts
    t0 = part * tn
    nrm = c.nrms[g]
    cols = slice((g * c.TPG + t0) * P, (g * c.TPG + t0 + tn) * P)
    for h, bt in ((0, c.BT0), (1, c.BT1)):
        nc.sync.dma_start_transpose(
            out=bt[:, cols].rearrange("q (t p) -> q t p", t=tn),
            in_=nrm[:, h, t0 * P:(t0 + tn) * P])
        if c.fp8:
            # fp8 cast for the DoubleRow matmuls (ACT Copy; every act
            # table set contains copy, so no table switch)
            nc.scalar.activation(out=c.BTf[:, h, cols], in_=bt[:, cols],
                                 func=AF.Copy)


def new_nrm(c, g):
    pool = c.keep_pool if g == 0 else c.nrm_pool
    tg = "keep" if g == 0 else "nrm"
    nrm = pool.tile([P, 2, c.TPG * P], BF16, tag=tg, name=f"nrm{g}")
    c.nrms[g] = nrm
    if g == 0:
        c.kept = nrm


def emit_chunk(c, i, q):
    """One (row-tile i, column-chunk q) unit: matmuls -> exp(+rowsum)
    -> colsum adds (minus the d=0 diagonal tile in chunk q=0)."""
    nc = c.nc
    CH = c.chunk
    c0 = i * P + q * CH
    ps = c.psum_pool.tile([P, CH], F32, tag="ps", name="mm_ps")
    NJ = CH // 512
    if c.fp8:
        lhs = c.BTf[:, :, i * P:(i + 1) * P]
        for j in range(NJ):
            nc.tensor.matmul(ps[:, j * 512:(j + 1) * 512], lhs,
                             c.BTf[:, :, c0 + j * 512:c0 + (j + 1) * 512],
                             start=True, stop=True, perf_mode=DR)
    else:
        lhs0 = c.BT0[:, i * P:(i + 1) * P]
        lhs1 = c.BT1[:, i * P:(i + 1) * P]
        for j in range(NJ):
            nc.tensor.matmul(ps[:, j * 512:(j + 1) * 512], lhs0,
                             c.BT0[:, c0 + j * 512:c0 + (j + 1) * 512],
                             start=True, stop=False)
        for j in range(NJ):
            nc.tensor.matmul(ps[:, j * 512:(j + 1) * 512], lhs1,
                             c.BT1[:, c0 + j * 512:c0 + (j + 1) * 512],
                             start=False, stop=True)
    if c.ablate_exp_cols == 0:
        return
    eo = c.exp_pool.tile([P, CH], BF16, tag="eo", name="eo")
    acc_col = c.rs_all[:, i * c.NQ + q: i * c.NQ + q + 1]
    ecols = c.ablate_exp_cols or CH
    nc.scalar.activation(out=eo[:, 0:ecols], in_=ps[:, 0:ecols],
                         func=AF.Exp, scale=2.0, accum_out=acc_col)
    if c.ablate_colsum:
        return
    if q == 0:
        nc.vector.tensor_tensor(
            out=c.acc[:, c0 + P:c0 + CH], in0=c.acc[:, c0 + P:c0 + CH],
            in1=eo[:, P:CH], op=ALU.add)
    else:
        nc.vector.tensor_tensor(
            out=c.acc[:, c0:c0 + CH], in0=c.acc[:, c0:c0 + CH],
            in1=eo[:], op=ALU.add)


def emit_csred(c, cst, cen):
    """Reduce acc[:, cst:cen] over the 128 partitions (ones-vector
    matmuls, 512-wide per PSUM-bank), stage via ACT Copy, DMA out."""
    nc = c.nc
    if cen <= cst:
        return
    for s0 in range(cst, cen, c.chunk):
        s1 = min(s0 + c.chunk, cen)
        red = c.psum_pool.tile([1, s1 - s0], F32, tag="ps", name="csred")
        for k in range(0, s1 - s0, 512):
            ke = min(k + 512, s1 - s0)
            nc.tensor.matmul(red[:, k:ke], c.ones[:],
                             c.acc[:, s0 + k:s0 + ke],
                             start=True, stop=True)
        stg = c.cso_pool.tile([1, s1 - s0], F32, tag="cso", name="cso")
        nc.scalar.activation(out=stg[:], in_=red[:], func=AF.Copy)
        nc.sync.dma_start(out=c.cs_dram.ap()[:, s0:s1], in_=stg[:])


def emit_d64(c):
    """Distance-D tiles (i, i+D), batched into one PSUM tile: row sums
    only (each unordered global pair is computed by exactly two cores)."""
    nc = c.nc
    MT, D_ = c.MT, c.D
    ps = c.psum_pool.tile([P, MT * P], F32, tag="ps", name="ps64")
    for i in range(MT):
        j0 = (i + D_) * P
        if c.fp8:
            nc.tensor.matmul(ps[:, i * P:(i + 1) * P],
                             c.BTf[:, :, i * P:(i + 1) * P],
                             c.BTf[:, :, j0:j0 + P],
                             start=True, stop=True, perf_mode=DR)
        else:
            nc.tensor.matmul(ps[:, i * P:(i + 1) * P],
                             c.BT0[:, i * P:(i + 1) * P],
                             c.BT0[:, j0:j0 + P], start=True, stop=False)
            nc.tensor.matmul(ps[:, i * P:(i + 1) * P],
                             c.BT1[:, i * P:(i + 1) * P],
                             c.BT1[:, j0:j0 + P], start=False, stop=True)
    # per-tile accum exps: the row sums land directly in rs64 on the ACT
    eo = c.exp_pool.tile([P, MT * P], BF16, tag="eo", name="eo64")
    for i in range(MT):
        nc.scalar.activation(out=eo[:, i * P:(i + 1) * P],
                             in_=ps[:, i * P:(i + 1) * P],
                             func=AF.Exp, scale=2.0,
                             accum_out=c.rs64[:, i:i + 1])


def emit_all(c):
    nc = c.nc
    c.slabs, c.nrms = {}, {}
    G, MT, NQ, TPG = c.G, c.MT, c.NQ, c.TPG

    # slabs 0..1: loads dispatched first, then sums of squares on the
    # (idle) ACT and Newton+normalize on the DVE, in half-slab pieces
    # with half-slab XBARs so the first chunks can start early.  Loads
    # 2..G-1 are dispatched right after (SP queue order keeps them off
    # the XBARs' critical path; their DMAs stream during pass q=0).
    HN = max(1, TPG // 2)
    nparts = 2 if TPG % 2 == 0 else 1
    for g in range(min(2, G)):
        emit_load(c, g, nparts=nparts)
    for g in range(min(2, G)):
        new_nrm(c, g)
        for hf in range(TPG // HN):
            emit_sq(c, g, hf * HN, HN)
            emit_norm(c, g, hf * HN, HN)
            emit_xbar(c, g, part=hf, nparts=TPG // HN)
    for g in range(2, G):
        emit_load(c, g)

    # remaining slab s is first needed at pass q = s-1: thread its
    # processing between the chunk emissions of pass q = s-2 (ACT sums
    # of squares in 4-tile pieces, then one batched Newton+normalize +
    # XBAR - the norms are cheap 4x-mode DVE ops).
    def inserts_for_pass(q):
        g = q + 2
        if g >= G:
            return {}
        new_nrm(c, g)
        # sq pieces start at i=5 so slab g's load (streaming during this
        # pass) has certainly landed - an ACT-queue wait on an unfinished
        # load would stall the exp stream behind it.
        nu = (TPG + 3) // 4
        ins = {5 + 2 * k: ("sq", g, k * 4, min(4, TPG - k * 4))
               for k in range(nu)}
        ins[5 + 2 * nu + 1] = ("fin", g, 0, TPG)
        return ins

    # colsum-reduce scheduling: segment [q*CH, (q+1)*CH) is final after
    # chunk (MT-1, q) (col c's last write is during pass floor(c/CH)).
    # Emitting the reduce right at the pass boundary injects a wait for
    # the pass's LAST colsum into the in-order PE queue, stalling the
    # next pass's matmuls - so defer each reduce into the next pass, and
    # stream the last pass's reduce in finalized pieces behind its
    # i-loop.
    CH = c.chunk
    NP = c.num_passes
    if NP < NQ or c.ablate_exp_cols == 0:
        # timing-probe build: some stat tiles are never fully written
        nc.vector.memset(c.pos_all[:], 0.0)
        nc.vector.memset(c.posA[:], 0.0)
        nc.vector.memset(c.rs64[:], 0.0)
        nc.vector.memset(c.rs_all[:], 0.0)
    for q in range(NP):
        ins = inserts_for_pass(q)
        for i in range(MT):
            emit_chunk(c, i, q)
            if i == 1 and q >= 1:
                emit_csred(c, (q - 1) * CH, q * CH)
            if q == NQ - 1 and MT > 14:
                if i in (6, 10, 14):
                    k = (i - 6) // 4
                    emit_csred(c, q * CH + k * 512, q * CH + (k + 1) * 512)
            if i in ins:
                kind, g, t0, tn = ins[i]
                if kind == "sq":
                    emit_sq(c, g, t0, tn)
                else:
                    emit_norm(c, g, t0, tn)
                    emit_xbar(c, g, part=0, nparts=1)
            if q == NQ - 1 and i == 2 and c.G > 1:
                emit_d64(c)
    if c.G == 1 and NP == NQ:
        emit_d64(c)
    done = (NQ - 1) * CH + (1536 if MT > 14 else 0)
    if NP == NQ:
        emit_csred(c, done, c.LT * P)
    else:
        emit_csred(c, (NP - 1) * CH, NP * CH)

    nc.vector.tensor_tensor(out=c.pos_all[:], in0=c.pos_all[:],
                            in1=c.posA[:], op=ALU.add)
    nc.sync.dma_start(out=c.rs_dram.ap(), in_=c.rs_all[:])
    nc.sync.dma_start(out=c.rs64_dram.ap(), in_=c.rs64[:])
    nc.sync.dma_start(out=c.pos_dram.ap(), in_=c.pos_all[:])


_CACHE = {}


def _get_program():
    if "nc" not in _CACHE:
        _CACHE["nc"] = build_program()
    return _CACHE["nc"]


def combine(rs, rs64, pos, cs, n_total=N_TOTAL, n_mine=N_MINE):
    """Host combine. rs: [C, P, MT*NQ], rs64: [C, P, MT], pos: [C,P,MT],
    cs: [C, P, LT*P]. Returns the scalar loss (f32)."""
    C = rs.shape[0]
    MT = n_mine // P
    NQ = rs.shape[2] // MT
    LT = cs.shape[2] // P
    denom = np.zeros(n_total, dtype=np.float64)
    posg = np.zeros(n_total, dtype=np.float64)
    for c in range(C):
        own = rs[c].reshape(P, MT, NQ).sum(axis=2) + rs64[c]   # [P, MT]
        sl = slice(c * n_mine, c * n_mine + n_mine)
        denom[sl] += own.T.reshape(-1)
        posg[sl] = pos[c].T.reshape(-1)
        colpart = cs[c].astype(np.float64).sum(axis=0)          # [LT*P]
        v = np.zeros(n_total)
        v[:LT * P] = colpart
        denom += np.roll(v, c * n_mine)
    denom -= np.exp(2.0)
    nt = np.log(denom) - 2.0 * posg
    return np.float32(nt.mean())


def kernel(x: np.ndarray, y: np.ndarray) -> np.ndarray:
    x = np.asarray(x, dtype=np.float32)
    y = np.asarray(y, dtype=np.float32)
    xy = np.concatenate([x, y], axis=0)

    import ml_dtypes
    nc, in_name, out_names = _get_program()
    in_maps = []
    for c in range(N_CORES):
        b_rot = np.ascontiguousarray(
            np.roll(xy, -c * N_MINE, axis=0)).astype(ml_dtypes.bfloat16)
        in_maps.append({in_name: b_rot})

    res = bass_utils.run_bass_kernel_spmd(
        nc, in_maps, core_ids=list(range(N_CORES)))
    g = lambda n: np.stack([np.asarray(res.results[c][n], dtype=np.float32)
                            for c in range(N_CORES)])
    return combine(g("rs"), g("rs64"), g("pos"), g("cs"))
